# revision 1
# baseline (speedup 1.0000x reference)
"""Trainium2 Bass kernel for nn_ModelSimplest (4D conv -> relu -> linear -> sigmoid).

Strategy: pure data parallel over batch (1024 -> 8 cores x 128).
The 4D conv is mapped onto TensorE matmuls:
  - contraction over the (k,l) input plane (324 values, split into 3 chunks
    of 108 partitions), expressed as a 2D-Toeplitz stationary matrix
    [108 x (3ch*6k'*6l' = 108)] built on the host from W4,
  - accumulation over the 169 (a,b) kernel offsets of the first two spatial
    dims (and the 3 chunks) in PSUM,
  - the moving operand streams (batch, j-window) columns: N = 64*6 = 384.
Epilogue fused on-chip: bias + relu (ScalarE, PSUM->SBUF bf16), then the
Linear(3888->1) as 36 accumulating [108x1] matmuls per batch tile, then
bias + sigmoid (ScalarE) and DMA out.

All data layout transforms / dtype casts are done host-side in numpy.
"""
import sys
from contextlib import ExitStack

import numpy as np

sys.path.insert(0, "/opt/trn_rl_repo")

from concourse import bacc, bass, mybir, tile  # noqa: E402
from concourse.bass_utils import run_bass_kernel_spmd  # noqa: E402

KK = 13      # conv kernel size per dim
S_IN = 18
S_OUT = 6
N_CORES = 8
B_TOTAL = 1024
B_CORE = B_TOTAL // N_CORES          # 128
B_SUB = 64                            # batch subtile per PSUM pass
N_SUB = B_CORE // B_SUB               # 2
NCH = 3
NPART = 108                           # partitions per contraction chunk
NM = NCH * S_OUT * S_OUT              # 108 output features per matmul
NCHUNK = 3                            # 324 = 3 * 108

F32 = mybir.dt.float32
BF16 = mybir.dt.bfloat16

_CACHE = {}


def _build_nc():
    nc = bacc.Bacc(None, target_bir_lowering=False)

    xp = nc.dram_tensor("xp", [NCHUNK, N_SUB, NPART, S_IN, S_IN, B_SUB],
                        BF16, kind="ExternalInput")
    tw = nc.dram_tensor("tw", [NCHUNK, KK, NPART, KK, NM], BF16,
                        kind="ExternalInput")
    wl = nc.dram_tensor("wl", [NPART, S_OUT * S_OUT], BF16,
                        kind="ExternalInput")
    bias4 = nc.dram_tensor("bias4", [NPART, 1], F32, kind="ExternalInput")
    blin = nc.dram_tensor("blin", [1, 1], F32, kind="ExternalInput")
    out = nc.dram_tensor("out", [1, B_CORE], F32, kind="ExternalOutput")

    with tile.TileContext(nc) as tc, ExitStack() as ctx:
        cpool = ctx.enter_context(tc.tile_pool(name="consts", bufs=1))
        wl_sb = cpool.tile([NPART, S_OUT * S_OUT], BF16)
        bias_sb = cpool.tile([NPART, 1], F32)
        blin_sb = cpool.tile([1, 1], F32)
        consts_loaded = []

        def load_consts():
            # issued after the first compute-critical DMAs
            if not consts_loaded:
                nc.sync.dma_start(wl_sb[:], wl[:])
                nc.sync.dma_start(bias_sb[:], bias4[:])
                nc.sync.dma_start(blin_sb[:], blin[:])
                consts_loaded.append(True)

        xpool = ctx.enter_context(tc.tile_pool(name="xs", bufs=2))
        twpool = ctx.enter_context(tc.tile_pool(name="tws", bufs=1))
        tw_tiles = {}

        def get_tw(c, a):
            # Lazily DMA each Toeplitz block right before first use so the
            # critical first x/tw transfers aren't queued behind all 39.
            if (c, a) not in tw_tiles:
                twt = twpool.tile([NPART, KK, NM], BF16, tag=f"tw{c}_{a}",
                                  name=f"tw{c}_{a}")
                nc.sync.dma_start(twt[:], tw[c, a])
                tw_tiles[(c, a)] = twt
            return tw_tiles[(c, a)]
        pspool = ctx.enter_context(
            tc.tile_pool(name="ps", bufs=1, space=bass.MemorySpace.PSUM))
        hpool = ctx.enter_context(tc.tile_pool(name="hs", bufs=1))
        opool = ctx.enter_context(tc.tile_pool(name="outs", bufs=2))

        pending = []

        def emit_epilogue():
            # bias+relu -> h ; linear -> logit psum; sigmoid; out DMA.
            # Deferred so it interleaves with the next tile's conv matmuls
            # instead of stalling the PE at the tile boundary.
            te, pse = pending.pop(0)
            lg = pspool.tile([1, B_SUB], F32, tag="lg", name=f"lg_{te}")
            for i in range(S_OUT):
                h = hpool.tile([NM, S_OUT, B_SUB], BF16, tag=f"h{i}",
                               name=f"h{i}_{te}")
                nc.scalar.activation(
                    h[:], pse[i][:],
                    mybir.ActivationFunctionType.Relu,
                    bias=bias_sb[:],
                )
                for j in range(S_OUT):
                    nc.tensor.matmul(
                        lg[:],
                        wl_sb[:, i * S_OUT + j:i * S_OUT + j + 1],
                        h[:, j, :],
                        start=(i == 0 and j == 0),
                        stop=(i == S_OUT - 1 and j == S_OUT - 1),
                    )
            ot = opool.tile([1, B_SUB], F32, tag="ot", name=f"ot_{te}")
            nc.scalar.activation(
                ot[:], lg[:],
                mybir.ActivationFunctionType.Sigmoid,
                bias=blin_sb[:],
            )
            nc.sync.dma_start(out[:, te * B_SUB:(te + 1) * B_SUB], ot[:])

        for t in range(N_SUB):
            ps = [
                pspool.tile([NM, S_OUT, B_SUB], F32, tag=f"ps{i}", name=f"ps{i}_{t}")
                for i in range(S_OUT)
            ]
            for c in range(NCHUNK):
                get_tw(c, 0)
                get_tw(c, 1)
                xts = []
                for ia in range(S_IN):
                    xia = xpool.tile([NPART, S_IN, B_SUB], BF16,
                                     tag=f"x{ia}", name=f"x{c}_{t}_{ia}")
                    nc.sync.dma_start(xia[:], xp[c, t, :, ia])
                    xts.append(xia)
                for a in range(KK):
                    twt = get_tw(c, a)
                    # prefetch next (c, a) block one step ahead
                    if a + 1 < KK:
                        get_tw(c, a + 1)
                    elif c + 1 < NCHUNK:
                        get_tw(c + 1, 0)
                    first = (c == 0 and a == 0)
                    last = (c == NCHUNK - 1 and a == KK - 1)
                    if c == 0 and a == 2:
                        load_consts()
                    if pending and c == 0 and a == 1:
                        emit_epilogue()
                    for boff in range(KK):
                        lhsT = twt[:, boff, :]
                        for i in range(S_OUT):
                            # rhs: [108, 6, b_sub] (jb window, batch inner)
                            rhs = xts[i + a][:, boff:boff + S_OUT, :]
                            nc.tensor.matmul(
                                ps[i][:],
                                lhsT,
                                rhs,
                                start=(first and boff == 0),
                                stop=(last and boff == KK - 1),
                            )
            pending.append((t, ps))

        while pending:
            emit_epilogue()

    nc.compile()
    return nc


def _prep_inputs(x, W4, b4, Wlin, blin):
    """Host-side layout transforms. Returns the shared (weight) arrays and
    the per-core x shards."""
    B = x.shape[0]
    # x_prep[c*108+p = k*18+l][ia][jb][b], bf16
    xt = np.ascontiguousarray(
        x[:, 0].transpose(3, 4, 1, 2, 0)).astype(jnp_bf16)
    x_prep = xt.reshape(NCHUNK, NPART, S_IN, S_IN, B)

    # T_flat[kl, a, boff, m]
    T_flat = np.zeros((324, KK, KK, NM), np.float32)
    kl = np.arange(324)
    k_in_v = kl // S_IN
    l_in_v = kl % S_IN
    W4t = W4[:, 0].transpose(0, 3, 4, 1, 2)  # [ch, dk, dl, a, boff]
    for ch in range(NCH):
        for kp in range(S_OUT):
            for lp in range(S_OUT):
                m = ch * 36 + kp * 6 + lp
                dk = k_in_v - kp
                dl = l_in_v - lp
                valid = (dk >= 0) & (dk < KK) & (dl >= 0) & (dl < KK)
                T_flat[valid, :, :, m] = W4t[ch, dk[valid], dl[valid]]
    # -> [c, a, p, boff, m]
    T_all = np.ascontiguousarray(
        T_flat.reshape(NCHUNK, NPART, KK, KK, NM).transpose(0, 2, 1, 3, 4))
    tw_np = T_all.astype(jnp_bf16)  # [c, a, p, boff, m]

    # wl[m, i*6+j] = Wlin[0, ch*1296 + i*216 + j*36 + (m%36)]
    m_idx = np.arange(NPART)
    ch_idx = m_idx // 36
    rem = m_idx % 36
    i_idx = np.arange(S_OUT)
    j_idx = np.arange(S_OUT)
    feat = (ch_idx[:, None, None] * 1296 + i_idx[None, :, None] * 216
            + j_idx[None, None, :] * 36 + rem[:, None, None])
    wl_np = Wlin[0, feat].reshape(NPART, S_OUT * S_OUT).astype(jnp_bf16)

    bias4_np = np.ascontiguousarray(
        b4[m_idx // 36].astype(np.float32).reshape(NPART, 1))
    blin_np = np.asarray(blin, np.float32).reshape(1, 1)
    return x_prep, tw_np, wl_np, bias4_np, blin_np


try:
    import ml_dtypes
    jnp_bf16 = ml_dtypes.bfloat16
except ImportError:  # pragma: no cover
    import jax.numpy as jnp
    jnp_bf16 = jnp.bfloat16


def kernel(x, W4, b4, Wlin, blin, _profile=False):
    x = np.asarray(x)
    W4 = np.asarray(W4)
    b4 = np.asarray(b4)
    Wlin = np.asarray(Wlin)
    blin = np.asarray(blin)

    x_prep, tw_np, wl_np, bias4_np, blin_np = _prep_inputs(
        x, W4, b4, Wlin, blin)

    if "nc" not in _CACHE:
        _CACHE["nc"] = _build_nc()
    nc = _CACHE["nc"]

    in_maps = []
    for core in range(N_CORES):
        b0 = core * B_CORE
        shard = x_prep[:, :, :, :, b0:b0 + B_CORE]
        shard = shard.reshape(NCHUNK, NPART, S_IN, S_IN, N_SUB, B_SUB)
        shard = np.ascontiguousarray(shard.transpose(0, 4, 1, 2, 3, 5))
        in_maps.append({
            "xp": shard,
            "tw": tw_np,
            "wl": wl_np,
            "bias4": bias4_np,
            "blin": blin_np,
        })

    res = run_bass_kernel_spmd(
        nc, in_maps, core_ids=list(range(N_CORES)), trace=_profile)
    outs = [res.results[i]["out"].reshape(B_CORE) for i in range(N_CORES)]
    full = np.concatenate(outs).reshape(B_TOTAL, 1).astype(np.float32)
    if _profile:
        return full, res
    return full



# revision 2
# speedup vs baseline: 1.8759x; 1.8759x over previous
"""Trainium2 Bass kernel for nn_ModelSimplest (4D conv -> relu -> linear -> sigmoid).

fp8 DoubleRow version. Data parallel over batch (1024 -> 8 cores x 128).

Conv mapping (per core, per 64-batch subtile):
  - contraction over the (k,l) input plane: 324 rows (k-major), split into
    three 108-row thirds tau0/tau1/tau2.
  - Toeplitz stationary built from W4 on the host: T[kl, a, boff, m] with
    m = (ch, ok, ol) in [0,108); W values scaled by 256 for fp8 range.
  - fp8e4 DoubleRow matmuls contract 2x108 rows at once:
      * per (a, boff): one DR matmul with halves (tau0, tau1)
      * per (a, boff-pair (2q,2q+1)): one DR matmul with halves
        (tau2 @ J-window 2q, tau2 @ J-window 2q+1) -- the second half comes
        from a J-shifted duplicate of the tau2 block in SBUF.
      * per a: one normal matmul for (boff=12, tau2).
    => 20 matmuls per (a, oi) instead of 39 108-row chunks.
  - accumulation over a in [0,13) in PSUM; 6 PSUM tiles (one per oi).
Epilogue fused on-chip: bias*256 + relu (ScalarE, PSUM->SBUF bf16), the
Linear(3888->1) as 36 accumulating [108x1] matmuls per batch tile with
Wlin/256, then bias + sigmoid (ScalarE) and DMA out.
"""
import sys
from contextlib import ExitStack

import numpy as np

sys.path.insert(0, "/opt/trn_rl_repo")

from concourse import bacc, bass, mybir, tile  # noqa: E402
from concourse.bass_utils import run_bass_kernel_spmd  # noqa: E402

KK = 13      # conv kernel size per dim
S_IN = 18
S_OUT = 6
N_CORES = 8
B_TOTAL = 1024
B_CORE = B_TOTAL // N_CORES          # 128
B_SUB = 64                            # batch subtile per PSUM pass
N_SUB = B_CORE // B_SUB               # 2
NCH = 3
NP = 108                              # partitions per third
NM = NCH * S_OUT * S_OUT              # 108 output features
NMP = 112                             # padded m stride (dim1 step % 16 == 0)
NPAIR = 6                             # boff pairs (0,1)..(10,11)
WSCALE = 256.0

F32 = mybir.dt.float32
BF16 = mybir.dt.bfloat16
FP8 = mybir.dt.float8e4
DR = mybir.MatmulPerfMode.DoubleRow

_CACHE = {}


def _build_nc():
    nc = bacc.Bacc(None, target_bir_lowering=False)

    # x tiles, host-prepared per (subtile, ia):
    #  xa: [p, g, j, b] = x[b, ia, j, kl=g*108+p]          (thirds 0,1)
    #  xq: [p, g, j, b] = x[b, ia, j+g, kl=216+p], j<17    (third 2, J-shifted dup)
    xa = nc.dram_tensor("xa", [N_SUB, S_IN, NP, 2, S_IN, B_SUB], FP8,
                        kind="ExternalInput")
    xq = nc.dram_tensor("xq", [N_SUB, S_IN, NP, 2, S_IN - 1, B_SUB], FP8,
                        kind="ExternalInput")
    # stationary Toeplitz blocks (fp8, scaled by WSCALE):
    tw01 = nc.dram_tensor("tw01", [KK, NP, KK, 2, NMP], FP8,
                          kind="ExternalInput")
    twpr = nc.dram_tensor("twpr", [KK, NP, NPAIR, 2, NMP], FP8,
                          kind="ExternalInput")
    twsg = nc.dram_tensor("twsg", [KK, NP, NMP], FP8, kind="ExternalInput")
    wl = nc.dram_tensor("wl", [NP, S_OUT * S_OUT], BF16,
                        kind="ExternalInput")
    bias4 = nc.dram_tensor("bias4", [NP, 1], F32, kind="ExternalInput")
    blin = nc.dram_tensor("blin", [1, 1], F32, kind="ExternalInput")
    out = nc.dram_tensor("out", [1, B_CORE], F32, kind="ExternalOutput")

    with tile.TileContext(nc) as tc, ExitStack() as ctx:
        cpool = ctx.enter_context(tc.tile_pool(name="consts", bufs=1))
        wl_sb = cpool.tile([NP, S_OUT * S_OUT], BF16)
        bias_sb = cpool.tile([NP, 1], F32)
        blin_sb = cpool.tile([1, 1], F32)
        consts_loaded = []

        def load_consts():
            if not consts_loaded:
                nc.sync.dma_start(wl_sb[:], wl[:])
                nc.sync.dma_start(bias_sb[:], bias4[:])
                nc.sync.dma_start(blin_sb[:], blin[:])
                consts_loaded.append(True)

        xpool = ctx.enter_context(tc.tile_pool(name="xs", bufs=1))
        twpool = ctx.enter_context(tc.tile_pool(name="tws", bufs=1))
        tw_tiles = {}

        def get_tw(a):
            # resident for the whole kernel; loaded lazily in a-order
            if a not in tw_tiles:
                t01 = twpool.tile([NP, KK, 2, NMP], FP8, tag=f"tw01_{a}",
                                  name=f"tw01_{a}")
                tpr = twpool.tile([NP, NPAIR, 2, NMP], FP8, tag=f"twpr_{a}",
                                  name=f"twpr_{a}")
                tsg = twpool.tile([NP, NMP], FP8, tag=f"twsg_{a}",
                                  name=f"twsg_{a}")
                nc.sync.dma_start(t01[:], tw01[a])
                nc.sync.dma_start(tpr[:], twpr[a])
                nc.sync.dma_start(tsg[:], twsg[a])
                tw_tiles[a] = (t01, tpr, tsg)
            return tw_tiles[a]

        pspool = ctx.enter_context(
            tc.tile_pool(name="ps", bufs=1, space=bass.MemorySpace.PSUM))
        hpool = ctx.enter_context(tc.tile_pool(name="hs", bufs=1))
        opool = ctx.enter_context(tc.tile_pool(name="outs", bufs=2))

        x_tiles = {}

        def load_x(t, ia):
            # one buffer slot per ia; subtile t+1's tile (t+1, ia) reuses
            # subtile t's slot (WAR dependency inserted by the pool).
            if (t, ia) in x_tiles:
                return
            xat = xpool.tile([NP, 2, S_IN, B_SUB], FP8,
                             tag=f"xa{ia}", name=f"xa_{t}_{ia}")
            xqt = xpool.tile([NP, 2, S_IN - 1, B_SUB], FP8,
                             tag=f"xq{ia}", name=f"xq_{t}_{ia}")
            nc.sync.dma_start(xat[:], xa[t, ia])
            nc.sync.dma_start(xqt[:], xq[t, ia])
            x_tiles[(t, ia)] = (xat, xqt)

        pending = []

        def emit_epilogue():
            te, pse = pending.pop(0)
            lg = pspool.tile([1, B_SUB], F32, tag="lg", name=f"lg_{te}")
            for i in range(S_OUT):
                h = hpool.tile([NM, S_OUT, B_SUB], BF16, tag=f"h{i}",
                               name=f"h{i}_{te}")
                nc.scalar.activation(
                    h[:], pse[i][:],
                    mybir.ActivationFunctionType.Relu,
                    bias=bias_sb[:],
                )
                for j in range(S_OUT):
                    nc.tensor.matmul(
                        lg[:],
                        wl_sb[:, i * S_OUT + j:i * S_OUT + j + 1],
                        h[:, j, :],
                        start=(i == 0 and j == 0),
                        stop=(i == S_OUT - 1 and j == S_OUT - 1),
                    )
            ot = opool.tile([1, B_SUB], F32, tag="ot", name=f"ot_{te}")
            nc.scalar.activation(
                ot[:], lg[:],
                mybir.ActivationFunctionType.Sigmoid,
                bias=blin_sb[:],
            )
            nc.sync.dma_start(out[:, te * B_SUB:(te + 1) * B_SUB], ot[:])

        for t in range(N_SUB):
            ps = [
                pspool.tile([NM, S_OUT, B_SUB], F32, tag=f"ps{i}",
                            name=f"ps{i}_{t}")
                for i in range(S_OUT)
            ]
            # x tiles for this subtile: first the compute-critical ia 0..5,
            # then the first Toeplitz blocks, then the rest.
            for ia in range(S_OUT):
                load_x(t, ia)
            get_tw(0)
            get_tw(1)
            for ia in range(S_OUT, S_IN):
                load_x(t, ia)
            for a in range(KK):
                t01, tpr, tsg = get_tw(a)
                if a + 1 < KK:
                    get_tw(a + 1)
                if a == 2:
                    load_consts()
                # prefetch next subtile's x tile once slot (t, a-1) is dead
                if t + 1 < N_SUB and a >= 1:
                    load_x(t + 1, a - 1)
                if pending and a == 1:
                    emit_epilogue()
                xs = [x_tiles[(t, a + oi)][0] for oi in range(S_OUT)]
                xqs = [x_tiles[(t, a + oi)][1] for oi in range(S_OUT)]
                first = (a == 0)
                for boff in range(KK):
                    lhsT = t01[:, boff, :, 0:NM]
                    for oi in range(S_OUT):
                        nc.tensor.matmul(
                            ps[oi][:],
                            lhsT,
                            xs[oi][:, :, boff:boff + S_OUT, :],
                            start=(first and boff == 0),
                            stop=False,
                            perf_mode=DR,
                        )
                for pr in range(NPAIR):
                    lhsT = tpr[:, pr, :, 0:NM]
                    for oi in range(S_OUT):
                        nc.tensor.matmul(
                            ps[oi][:],
                            lhsT,
                            xqs[oi][:, :, 2 * pr:2 * pr + S_OUT, :],
                            start=False,
                            stop=False,
                            perf_mode=DR,
                        )
                last = (a == KK - 1)
                lhsT = tsg[:, 0:NM]
                for oi in range(S_OUT):
                    # boff=12, tau2: J-window [12,18) via the g=1 (J=j+1) half
                    nc.tensor.matmul(
                        ps[oi][:],
                        lhsT,
                        xqs[oi][:, 1, 11:17, :],
                        start=False,
                        stop=last,
                    )
            pending.append((t, ps))

        while pending:
            emit_epilogue()

    nc.compile()
    return nc


try:
    import ml_dtypes
    np_bf16 = ml_dtypes.bfloat16
    np_fp8 = ml_dtypes.float8_e4m3
except ImportError:  # pragma: no cover
    raise


def _prep_inputs(x, W4, b4, Wlin, blin):
    B = x.shape[0]
    # xt[kl, ia, j, b], kl = k*18 + l
    xt = np.ascontiguousarray(
        x[:, 0].transpose(3, 4, 1, 2, 0)).reshape(324, S_IN, S_IN, B)
    xt8 = xt.astype(np_fp8)

    # xa[ia, p, g, j, b] = xt[g*108+p, ia, j, b]
    xa_full = np.ascontiguousarray(
        xt8[:216].reshape(2, NP, S_IN, S_IN, B).transpose(2, 1, 0, 3, 4))
    # xq[ia, p, g, j, b] = xt[216+p, ia, j+g, b], j in [0,17)
    t2 = xt8[216:]  # [108, 18, 18, B]
    xq_full = np.empty((S_IN, NP, 2, S_IN - 1, B), np_fp8)
    xq_full[:, :, 0] = t2[:, :, :17].transpose(1, 0, 2, 3)
    xq_full[:, :, 1] = t2[:, :, 1:].transpose(1, 0, 2, 3)

    # T_flat[kl, a, boff, m] (fp32), scaled
    T_flat = np.zeros((324, KK, KK, NM), np.float32)
    kl = np.arange(324)
    k_in_v = kl // S_IN
    l_in_v = kl % S_IN
    W4t = W4[:, 0].transpose(0, 3, 4, 1, 2)  # [ch, dk, dl, a, boff]
    for ch in range(NCH):
        for kp in range(S_OUT):
            for lp in range(S_OUT):
                m = ch * 36 + kp * 6 + lp
                dk = k_in_v - kp
                dl = l_in_v - lp
                valid = (dk >= 0) & (dk < KK) & (dl >= 0) & (dl < KK)
                T_flat[valid, :, :, m] = W4t[ch, dk[valid], dl[valid]]
    Tq = (T_flat * WSCALE).astype(np_fp8)  # [324, 13, 13, 108]

    # tw01[a, p, boff, g, m] = Tq[g*108+p, a, boff, m]
    tw01_np = np.zeros((KK, NP, KK, 2, NMP), np_fp8)
    tw01_np[:, :, :, :, :NM] = Tq[:216].reshape(
        2, NP, KK, KK, NM).transpose(2, 1, 3, 0, 4)
    # twpr[a, p, q, g, m] = Tq[216+p, a, 2q+g, m]
    twpr_np = np.zeros((KK, NP, NPAIR, 2, NMP), np_fp8)
    twpr_np[:, :, :, :, :NM] = Tq[216:, :, :12].reshape(
        NP, KK, NPAIR, 2, NM).transpose(1, 0, 2, 3, 4)
    # twsg[a, p, m] = Tq[216+p, a, 12, m]
    twsg_np = np.zeros((KK, NP, NMP), np_fp8)
    twsg_np[:, :, :NM] = Tq[216:, :, 12].transpose(1, 0, 2)

    # wl[m, i*6+j] = Wlin[0, ch*1296 + i*216 + j*36 + (m%36)] / WSCALE
    m_idx = np.arange(NP)
    ch_idx = m_idx // 36
    rem = m_idx % 36
    i_idx = np.arange(S_OUT)
    j_idx = np.arange(S_OUT)
    feat = (ch_idx[:, None, None] * 1296 + i_idx[None, :, None] * 216
            + j_idx[None, None, :] * 36 + rem[:, None, None])
    wl_np = (Wlin[0, feat].reshape(NP, S_OUT * S_OUT)
             / WSCALE).astype(np_bf16)

    bias4_np = np.ascontiguousarray(
        (b4[m_idx // 36] * WSCALE).astype(np.float32).reshape(NP, 1))
    blin_np = np.asarray(blin, np.float32).reshape(1, 1)
    return xa_full, xq_full, tw01_np, twpr_np, twsg_np, wl_np, bias4_np, blin_np


def kernel(x, W4, b4, Wlin, blin, _profile=False):
    x = np.asarray(x)
    W4 = np.asarray(W4)
    b4 = np.asarray(b4)
    Wlin = np.asarray(Wlin)
    blin = np.asarray(blin)

    (xa_full, xq_full, tw01_np, twpr_np, twsg_np, wl_np, bias4_np,
     blin_np) = _prep_inputs(x, W4, b4, Wlin, blin)

    if "nc" not in _CACHE:
        _CACHE["nc"] = _build_nc()
    nc = _CACHE["nc"]

    in_maps = []
    for core in range(N_CORES):
        b0 = core * B_CORE
        xac = xa_full[:, :, :, :, b0:b0 + B_CORE].reshape(
            S_IN, NP, 2, S_IN, N_SUB, B_SUB)
        xac = np.ascontiguousarray(xac.transpose(4, 0, 1, 2, 3, 5))
        xqc = xq_full[:, :, :, :, b0:b0 + B_CORE].reshape(
            S_IN, NP, 2, S_IN - 1, N_SUB, B_SUB)
        xqc = np.ascontiguousarray(xqc.transpose(4, 0, 1, 2, 3, 5))
        in_maps.append({
            "xa": xac,
            "xq": xqc,
            "tw01": tw01_np,
            "twpr": twpr_np,
            "twsg": twsg_np,
            "wl": wl_np,
            "bias4": bias4_np,
            "blin": blin_np,
        })

    res = run_bass_kernel_spmd(
        nc, in_maps, core_ids=list(range(N_CORES)), trace=_profile)
    outs = [res.results[i]["out"].reshape(B_CORE) for i in range(N_CORES)]
    full = np.concatenate(outs).reshape(B_TOTAL, 1).astype(np.float32)
    if _profile:
        return full, res
    return full


# revision 3
# speedup vs baseline: 2.0628x; 1.0997x over previous
"""Trainium2 Bass kernel for nn_ModelSimplest (4D conv -> relu -> linear -> sigmoid).

fp8 DoubleRow, folded-boff version. Data parallel over batch (1024 -> 8x128).

Per (a, oi) the conv contracts rows r = (boff, k, l) in [13*324 = 4212].
The per-row J-shift (boff) is baked into the SBUF x tiles by the host:
partition slot (p, u, g) holds x[b, ia, j' + boff(r), k(r), l(r)], j' in
[0,6).  Rows are packed into 16 full fp8-DoubleRow matmuls of 256 rows
plus 1 tail DR matmul of 116 rows => 17 matmuls per (a, oi) streaming
384 columns each.  PSUM accumulates over a in [0,13); 6 PSUM tiles (oi).

Epilogue fused on-chip: bias*256 + relu (ScalarE -> bf16), Linear(3888->1)
as 36 accumulating [108x1] matmuls per batch tile with Wlin/256, sigmoid
(ScalarE), DMA out.
"""
import sys
from contextlib import ExitStack

import numpy as np

sys.path.insert(0, "/opt/trn_rl_repo")

from concourse import bacc, bass, mybir, tile  # noqa: E402
from concourse.bass_utils import run_bass_kernel_spmd  # noqa: E402

KK = 13
S_IN = 18
S_OUT = 6
N_CORES = 8
B_TOTAL = 1024
B_CORE = B_TOTAL // N_CORES          # 128
B_SUB = 64
N_SUB = B_CORE // B_SUB               # 2
NCH = 3
NM = NCH * S_OUT * S_OUT              # 108
NMP = 112                             # padded m stride (dim step % 16 == 0)
NROW = KK * S_IN * S_IN               # 4212 contraction rows per (a, oi)
NU = 17                               # DR units: 16 full + 1 tail
TAILP = (NROW - 16 * 256) // 2        # 58 partitions in tail unit
WSCALE = 256.0
NSLOT = 10                            # x tile slot rotation

F32 = mybir.dt.float32
BF16 = mybir.dt.bfloat16
FP8 = mybir.dt.float8e4
DR = mybir.MatmulPerfMode.DoubleRow

_CACHE = {}


def _row_maps():
    """slot (u, g, p) -> row r = boff*324 + kl; tail masked with -1."""
    rows = np.full((NU, 2, 128), -1, np.int64)
    r = np.arange(16 * 256)
    rows[:16] = r.reshape(16, 2, 128)
    rt = 16 * 256 + np.arange(NROW - 16 * 256)
    rows[16, :, :TAILP] = rt.reshape(2, TAILP)
    return rows


def _build_nc():
    nc = bacc.Bacc(None, target_bir_lowering=False)

    # xf[t, ia, p, u, g, j, b]
    xf = nc.dram_tensor("xf", [N_SUB, S_IN, 128, NU, 2, S_OUT, B_SUB], FP8,
                        kind="ExternalInput")
    # tf[a, p, u, g, m]
    tf = nc.dram_tensor("tf", [KK, 128, NU, 2, NMP], FP8,
                        kind="ExternalInput")
    wl = nc.dram_tensor("wl", [NM, S_OUT * S_OUT], BF16, kind="ExternalInput")
    bias4 = nc.dram_tensor("bias4", [NM, 1], F32, kind="ExternalInput")
    blin = nc.dram_tensor("blin", [1, 1], F32, kind="ExternalInput")
    out = nc.dram_tensor("out", [1, B_CORE], F32, kind="ExternalOutput")

    with tile.TileContext(nc) as tc, ExitStack() as ctx:
        cpool = ctx.enter_context(tc.tile_pool(name="consts", bufs=1))
        wl_sb = cpool.tile([NM, S_OUT * S_OUT], BF16)
        bias_sb = cpool.tile([NM, 1], F32)
        blin_sb = cpool.tile([1, 1], F32)
        consts_loaded = []

        def load_consts():
            if not consts_loaded:
                nc.sync.dma_start(wl_sb[:], wl[:])
                nc.sync.dma_start(bias_sb[:], bias4[:])
                nc.sync.dma_start(blin_sb[:], blin[:])
                consts_loaded.append(True)

        xpool = ctx.enter_context(tc.tile_pool(name="xs", bufs=1))
        twpool = ctx.enter_context(tc.tile_pool(name="tws", bufs=1))
        tw_tiles = {}

        def get_tw(a):
            if a not in tw_tiles:
                twt = twpool.tile([128, NU, 2, NMP], FP8, tag=f"tf{a}",
                                  name=f"tf{a}")
                nc.sync.dma_start(twt[:], tf[a])
                tw_tiles[a] = twt
            return tw_tiles[a]

        x_tiles = {}

        def load_x(t, ia):
            if (t, ia) in x_tiles:
                return
            xt = xpool.tile([128, NU, 2, S_OUT, B_SUB], FP8,
                            tag=f"x{ia % NSLOT}", name=f"x_{t}_{ia}")
            nc.sync.dma_start(xt[:], xf[t, ia])
            x_tiles[(t, ia)] = xt

        pspool = ctx.enter_context(
            tc.tile_pool(name="ps", bufs=1, space=bass.MemorySpace.PSUM))
        hpool = ctx.enter_context(tc.tile_pool(name="hs", bufs=1))
        opool = ctx.enter_context(tc.tile_pool(name="outs", bufs=2))

        pending = []

        def emit_epilogue():
            te, pse = pending.pop(0)
            lg = pspool.tile([1, B_SUB], F32, tag="lg", name=f"lg_{te}")
            for i in range(S_OUT):
                h = hpool.tile([NM, S_OUT, B_SUB], BF16, tag=f"h{i}",
                               name=f"h{i}_{te}")
                nc.scalar.activation(
                    h[:], pse[i][:],
                    mybir.ActivationFunctionType.Relu,
                    bias=bias_sb[:],
                )
                for j in range(S_OUT):
                    nc.tensor.matmul(
                        lg[:],
                        wl_sb[:, i * S_OUT + j:i * S_OUT + j + 1],
                        h[:, j, :],
                        start=(i == 0 and j == 0),
                        stop=(i == S_OUT - 1 and j == S_OUT - 1),
                    )
            ot = opool.tile([1, B_SUB], F32, tag="ot", name=f"ot_{te}")
            nc.scalar.activation(
                ot[:], lg[:],
                mybir.ActivationFunctionType.Sigmoid,
                bias=blin_sb[:],
            )
            nc.sync.dma_start(out[:, te * B_SUB:(te + 1) * B_SUB], ot[:])

        for t in range(N_SUB):
            ps = [
                pspool.tile([NM, S_OUT, B_SUB], F32, tag=f"ps{i}",
                            name=f"ps{i}_{t}")
                for i in range(S_OUT)
            ]
            for ia in range(S_OUT):
                load_x(t, ia)
            get_tw(0)
            get_tw(1)
            for ia in range(S_OUT, NSLOT):
                load_x(t, ia)
            for a in range(KK):
                twt = get_tw(a)
                if a + 1 < KK:
                    get_tw(a + 1)
                if a == 2:
                    load_consts()
                # rotate in the x tile whose slot just freed (slot (a-1)%NSLOT)
                nxt = a - 1 + NSLOT
                if a >= 1:
                    if nxt < S_IN:
                        load_x(t, nxt)
                    elif t + 1 < N_SUB:
                        load_x(t + 1, nxt - S_IN)
                if pending and a == 1:
                    emit_epilogue()
                xs = [x_tiles[(t, a + oi)] for oi in range(S_OUT)]
                for u in range(NU):
                    if u < 16:
                        lhsT = twt[:, u, :, 0:NM]
                    else:
                        lhsT = twt[0:TAILP, u, :, 0:NM]
                    for oi in range(S_OUT):
                        if u < 16:
                            rhs = xs[oi][:, u, :, :, :]
                        else:
                            rhs = xs[oi][0:TAILP, u, :, :, :]
                        nc.tensor.matmul(
                            ps[oi][:],
                            lhsT,
                            rhs,
                            start=(a == 0 and u == 0),
                            stop=(a == KK - 1 and u == NU - 1),
                            perf_mode=DR,
                        )
            pending.append((t, ps))

        while pending:
            emit_epilogue()

    nc.compile()
    return nc


try:
    import ml_dtypes
    np_bf16 = ml_dtypes.bfloat16
    np_fp8 = ml_dtypes.float8_e4m3
except ImportError:  # pragma: no cover
    raise


def _prep_inputs(x, W4, b4, Wlin, blin):
    B = x.shape[0]
    rows = _row_maps()                    # [u, g, p] -> r or -1
    rmask = rows >= 0
    rsafe = np.where(rmask, rows, 0)
    boff_r = rsafe // 324                 # [u, g, p]
    kl_r = rsafe % 324

    # xt[kl, ia, j, b]
    xt = np.ascontiguousarray(
        x[:, 0].transpose(3, 4, 1, 2, 0)).reshape(324, S_IN, S_IN, B)
    xt8 = xt.astype(np_fp8)

    # xf_all[u, g, p, j, ia, b] = xt8[kl_r, ia, boff_r + j, b]
    jj = boff_r[..., None] + np.arange(S_OUT)          # [u, g, p, j]
    xf_all = xt8[kl_r[..., None], :, jj, :]            # [u, g, p, j, ia, B]
    xf_all *= rmask[..., None, None, None].astype(np_fp8)
    # -> [ia, p, u, g, j, B]
    xf_all = np.ascontiguousarray(xf_all.transpose(4, 2, 0, 1, 3, 5))

    # T_flat[kl, a, boff, m]
    T_flat = np.zeros((324, KK, KK, NM), np.float32)
    kl = np.arange(324)
    k_in_v = kl // S_IN
    l_in_v = kl % S_IN
    W4t = W4[:, 0].transpose(0, 3, 4, 1, 2)  # [ch, dk, dl, a, boff]
    for ch in range(NCH):
        for kp in range(S_OUT):
            for lp in range(S_OUT):
                m = ch * 36 + kp * 6 + lp
                dk = k_in_v - kp
                dl = l_in_v - lp
                valid = (dk >= 0) & (dk < KK) & (dl >= 0) & (dl < KK)
                T_flat[valid, :, :, m] = W4t[ch, dk[valid], dl[valid]]
    Tq = (T_flat * WSCALE).astype(np_fp8)    # [kl, a, boff, m]

    # tf[a, p, u, g, m] = Tq[kl_r, a, boff_r, m]
    tf_np = np.zeros((KK, 128, NU, 2, NMP), np_fp8)
    tgt = Tq[kl_r, :, boff_r, :]             # [u, g, p, a, m]
    tgt *= rmask[..., None, None].astype(np_fp8)
    tf_np[:, :, :, :, :NM] = tgt.transpose(3, 2, 0, 1, 4)

    m_idx = np.arange(NM)
    ch_idx = m_idx // 36
    rem = m_idx % 36
    i_idx = np.arange(S_OUT)
    j_idx = np.arange(S_OUT)
    feat = (ch_idx[:, None, None] * 1296 + i_idx[None, :, None] * 216
            + j_idx[None, None, :] * 36 + rem[:, None, None])
    wl_np = (Wlin[0, feat].reshape(NM, S_OUT * S_OUT)
             / WSCALE).astype(np_bf16)

    bias4_np = np.ascontiguousarray(
        (b4[m_idx // 36] * WSCALE).astype(np.float32).reshape(NM, 1))
    blin_np = np.asarray(blin, np.float32).reshape(1, 1)
    return xf_all, tf_np, wl_np, bias4_np, blin_np


def kernel(x, W4, b4, Wlin, blin, _profile=False):
    x = np.asarray(x)
    W4 = np.asarray(W4)
    b4 = np.asarray(b4)
    Wlin = np.asarray(Wlin)
    blin = np.asarray(blin)

    xf_all, tf_np, wl_np, bias4_np, blin_np = _prep_inputs(
        x, W4, b4, Wlin, blin)

    if "nc" not in _CACHE:
        _CACHE["nc"] = _build_nc()
    nc = _CACHE["nc"]

    in_maps = []
    for core in range(N_CORES):
        b0 = core * B_CORE
        # [ia, p, u, g, j, B] -> [t, ia, p, u, g, j, b]
        xc = xf_all[..., b0:b0 + B_CORE].reshape(
            S_IN, 128, NU, 2, S_OUT, N_SUB, B_SUB)
        xc = np.ascontiguousarray(xc.transpose(5, 0, 1, 2, 3, 4, 6))
        in_maps.append({
            "xf": xc,
            "tf": tf_np,
            "wl": wl_np,
            "bias4": bias4_np,
            "blin": blin_np,
        })

    res = run_bass_kernel_spmd(
        nc, in_maps, core_ids=list(range(N_CORES)), trace=_profile)
    outs = [res.results[i]["out"].reshape(B_CORE) for i in range(N_CORES)]
    full = np.concatenate(outs).reshape(B_TOTAL, 1).astype(np.float32)
    if _profile:
        return full, res
    return full


# revision 4
# speedup vs baseline: 2.1861x; 1.0598x over previous
"""Trainium2 Bass kernel for nn_ModelSimplest (4D conv -> relu -> linear -> sigmoid).

fp8 DoubleRow, folded-boff + wavefront ramp + a-paired tails.

Per (a, oi): contraction rows r = (boff, k, l) in [4212], J-shift baked into
SBUF tiles.  16 full 256-row DR matmuls (rows 0..4095) per (a, oi), plus the
116-row tail (boff=12, kl>=208) handled as DR matmuls pairing (a, a+1):
g=0 half reads x tile ia, g=1 half reads the NEXT ia's rows which are
duplicated into the same tile block.  215 matmuls per (t, oi).

Schedule: per subtile, a wavefront ramp over cells (a, oi) with a+oi<=4 so
the first matmul only needs tf[0] + x(t,0), then the standard a-major loop.
"""
import sys
from contextlib import ExitStack

import numpy as np

sys.path.insert(0, "/opt/trn_rl_repo")

from concourse import bacc, bass, mybir, tile  # noqa: E402
from concourse.bass_utils import run_bass_kernel_spmd  # noqa: E402

KK = 13
S_IN = 18
S_OUT = 6
N_CORES = 8
B_TOTAL = 1024
B_CORE = B_TOTAL // N_CORES          # 128
B_SUB = 64
N_SUB = B_CORE // B_SUB               # 2
NCH = 3
NM = NCH * S_OUT * S_OUT              # 108
NMP = 112
NROW = KK * S_IN * S_IN               # 4212
NU = 16                               # full 256-row DR units
NTAIL = NROW - NU * 256               # 116 tail rows (boff=12, kl 208..323)
NPAIR = 7                             # a-pairs (0,1)..(10,11) + (12,-)
WSCALE = 256.0
NSLOT = 10
RAMP_W = 5                            # wavefront cells with a+oi < RAMP_W

F32 = mybir.dt.float32
BF16 = mybir.dt.bfloat16
FP8 = mybir.dt.float8e4
DR = mybir.MatmulPerfMode.DoubleRow

_CACHE = {}


def _build_nc():
    nc = bacc.Bacc(None, target_bir_lowering=False)

    # xm[t, ia, p, u, g, j, b] : main rows r = u*256 + g*128 + p
    xm = nc.dram_tensor("xm", [N_SUB, S_IN, 128, NU, 2, S_OUT, B_SUB], FP8,
                        kind="ExternalInput")
    # xt[t, ia, p, g, j, b] : tail rows, g=0 from ia, g=1 from ia+1 (0 pad)
    xtl = nc.dram_tensor("xtl", [N_SUB, S_IN, NTAIL, 2, S_OUT, B_SUB], FP8,
                         kind="ExternalInput")
    # tfm[a, p, u, g, m]
    tfm = nc.dram_tensor("tfm", [KK, 128, NU, 2, NMP], FP8,
                         kind="ExternalInput")
    # tft[q, p, g, m] : tail pair (a=2q, a=2q+1); q=6 has g=1 zeros
    tft = nc.dram_tensor("tft", [NPAIR, NTAIL, 2, NMP], FP8,
                         kind="ExternalInput")
    wl = nc.dram_tensor("wl", [NM, S_OUT * S_OUT], BF16, kind="ExternalInput")
    bias4 = nc.dram_tensor("bias4", [NM, 1], F32, kind="ExternalInput")
    blin = nc.dram_tensor("blin", [1, 1], F32, kind="ExternalInput")
    out = nc.dram_tensor("out", [1, B_CORE], F32, kind="ExternalOutput")

    with tile.TileContext(nc) as tc, ExitStack() as ctx:
        cpool = ctx.enter_context(tc.tile_pool(name="consts", bufs=1))
        wl_sb = cpool.tile([NM, S_OUT * S_OUT], BF16)
        bias_sb = cpool.tile([NM, 1], F32)
        blin_sb = cpool.tile([1, 1], F32)
        consts_loaded = []

        def load_consts():
            if not consts_loaded:
                nc.sync.dma_start(wl_sb[:], wl[:])
                nc.sync.dma_start(bias_sb[:], bias4[:])
                nc.sync.dma_start(blin_sb[:], blin[:])
                consts_loaded.append(True)

        xpool = ctx.enter_context(tc.tile_pool(name="xs", bufs=1))
        twpool = ctx.enter_context(tc.tile_pool(name="tws", bufs=1))
        tw_tiles = {}
        tt_tiles = {}

        def get_tw(a):
            if a not in tw_tiles:
                twt = twpool.tile([128, NU, 2, NMP], FP8, tag=f"tfm{a}",
                                  name=f"tfm{a}")
                nc.sync.dma_start(twt[:], tfm[a])
                tw_tiles[a] = twt
                if a % 2 == 0:
                    q = a // 2
                    ttt = twpool.tile([NTAIL, 2, NMP], FP8, tag=f"tft{q}",
                                      name=f"tft{q}")
                    nc.sync.dma_start(ttt[:], tft[q])
                    tt_tiles[q] = ttt
            return tw_tiles[a]

        x_tiles = {}

        def load_x(t, ia):
            if (t, ia) in x_tiles:
                return
            xmt = xpool.tile([128, NU, 2, S_OUT, B_SUB], FP8,
                             tag=f"xm{ia % NSLOT}", name=f"xm_{t}_{ia}")
            xtt = xpool.tile([NTAIL, 2, S_OUT, B_SUB], FP8,
                             tag=f"xt{ia % NSLOT}", name=f"xt_{t}_{ia}")
            nc.sync.dma_start(xmt[:], xm[t, ia])
            nc.sync.dma_start(xtt[:], xtl[t, ia])
            x_tiles[(t, ia)] = (xmt, xtt)

        pspool = ctx.enter_context(
            tc.tile_pool(name="ps", bufs=1, space=bass.MemorySpace.PSUM))
        hpool = ctx.enter_context(tc.tile_pool(name="hs", bufs=1))
        opool = ctx.enter_context(tc.tile_pool(name="outs", bufs=2))

        pending = []

        def emit_epilogue():
            te, pse = pending.pop(0)
            lg = pspool.tile([1, B_SUB], F32, tag="lg", name=f"lg_{te}")
            for i in range(S_OUT):
                h = hpool.tile([NM, S_OUT, B_SUB], BF16, tag=f"h{i}",
                               name=f"h{i}_{te}")
                nc.scalar.activation(
                    h[:], pse[i][:],
                    mybir.ActivationFunctionType.Relu,
                    bias=bias_sb[:],
                )
                for j in range(S_OUT):
                    nc.tensor.matmul(
                        lg[:],
                        wl_sb[:, i * S_OUT + j:i * S_OUT + j + 1],
                        h[:, j, :],
                        start=(i == 0 and j == 0),
                        stop=(i == S_OUT - 1 and j == S_OUT - 1),
                    )
            ot = opool.tile([1, B_SUB], F32, tag="ot", name=f"ot_{te}")
            nc.scalar.activation(
                ot[:], lg[:],
                mybir.ActivationFunctionType.Sigmoid,
                bias=blin_sb[:],
            )
            nc.sync.dma_start(out[:, te * B_SUB:(te + 1) * B_SUB], ot[:])

        def cell(t, a, oi, ps):
            """All matmuls of cell (a, oi): 16 main units + tail pair."""
            xmt, xtt = x_tiles[(t, a + oi)]
            twt = get_tw(a)
            for u in range(NU):
                nc.tensor.matmul(
                    ps[oi][:],
                    twt[:, u, :, 0:NM],
                    xmt[:, u, :, :, :],
                    start=(a == 0 and u == 0),
                    stop=False,
                    perf_mode=DR,
                )
            if a % 2 == 0:
                nc.tensor.matmul(
                    ps[oi][:],
                    tt_tiles[a // 2][:, :, 0:NM],
                    xtt[:, :, :, :],
                    start=False,
                    stop=(a == KK - 1),
                    perf_mode=DR,
                )

        for t in range(N_SUB):
            ps = [
                pspool.tile([NM, S_OUT, B_SUB], F32, tag=f"ps{i}",
                            name=f"ps{i}_{t}")
                for i in range(S_OUT)
            ]
            get_tw(0)
            load_x(t, 0)
            get_tw(1)
            for ia in range(1, S_OUT):
                load_x(t, ia)
            for ia in range(S_OUT, NSLOT):
                load_x(t, ia)
            # wavefront ramp: cells with a + oi < RAMP_W
            for w in range(RAMP_W):
                get_tw(w + 1)  # prefetch stationaries one wavefront ahead
                for a in range(w + 1):
                    cell(t, a, w - a, ps)
                if w == 2:
                    load_consts()
                if w == 4 and pending:
                    emit_epilogue()
            # main loop
            for a in range(KK):
                get_tw(a)
                if a + 1 < KK:
                    get_tw(a + 1)
                nxt = a - 1 + NSLOT
                if a >= 1:
                    if nxt < S_IN:
                        load_x(t, nxt)
                    elif t + 1 < N_SUB:
                        load_x(t + 1, nxt - S_IN)
                for oi in range(S_OUT):
                    if a + oi >= RAMP_W:
                        cell(t, a, oi, ps)
            pending.append((t, ps))

        while pending:
            emit_epilogue()

    nc.compile()
    return nc


try:
    import ml_dtypes
    np_bf16 = ml_dtypes.bfloat16
    np_fp8 = ml_dtypes.float8_e4m3
except ImportError:  # pragma: no cover
    raise


def _prep_inputs(x, W4, b4, Wlin, blin):
    B = x.shape[0]
    # main rows r = u*256 + g*128 + p, r = boff*324 + kl
    r_main = np.arange(NU * 256).reshape(NU, 2, 128)
    boff_m = r_main // 324
    kl_m = r_main % 324
    # tail rows: boff=12, kl = 208 + p
    kl_t = 208 + np.arange(NTAIL)

    xt = np.ascontiguousarray(
        x[:, 0].transpose(3, 4, 1, 2, 0)).reshape(324, S_IN, S_IN, B)
    xt8 = xt.astype(np_fp8)

    # xm_all[u, g, p, j, ia, B] = xt8[kl_m, ia, boff_m + j, b]
    jj = boff_m[..., None] + np.arange(S_OUT)          # [u, g, p, j]
    xm_all = xt8[kl_m[..., None], :, jj, :]            # [u, g, p, j, ia, B]
    # -> [ia, p, u, g, j, B]
    xm_all = np.ascontiguousarray(xm_all.transpose(4, 2, 0, 1, 3, 5))

    # xtl_all[ia, p, g, j, B]: g=0: xt8[kl_t, ia, 12+j]; g=1: same from ia+1
    base = xt8[kl_t, :, 12:12 + S_OUT, :]              # [p, ia, j, B]
    base = base.transpose(1, 0, 2, 3)                  # [ia, p, j, B]
    xtl_all = np.zeros((S_IN, NTAIL, 2, S_OUT, B), np_fp8)
    xtl_all[:, :, 0] = base
    xtl_all[:S_IN - 1, :, 1] = base[1:]

    # T_flat[kl, a, boff, m]
    T_flat = np.zeros((324, KK, KK, NM), np.float32)
    kl = np.arange(324)
    k_in_v = kl // S_IN
    l_in_v = kl % S_IN
    W4t = W4[:, 0].transpose(0, 3, 4, 1, 2)  # [ch, dk, dl, a, boff]
    for ch in range(NCH):
        for kp in range(S_OUT):
            for lp in range(S_OUT):
                m = ch * 36 + kp * 6 + lp
                dk = k_in_v - kp
                dl = l_in_v - lp
                valid = (dk >= 0) & (dk < KK) & (dl >= 0) & (dl < KK)
                T_flat[valid, :, :, m] = W4t[ch, dk[valid], dl[valid]]
    Tq = (T_flat * WSCALE).astype(np_fp8)    # [kl, a, boff, m]

    # tfm[a, p, u, g, m] = Tq[kl_m, a, boff_m, m]
    tfm_np = np.zeros((KK, 128, NU, 2, NMP), np_fp8)
    tgt = Tq[kl_m, :, boff_m, :]             # [u, g, p, a, m]
    tfm_np[:, :, :, :, :NM] = tgt.transpose(3, 2, 0, 1, 4)

    # tft[q, p, g, m]: g=0: Tq[kl_t, 2q, 12, m]; g=1: Tq[kl_t, 2q+1, 12, m]
    tft_np = np.zeros((NPAIR, NTAIL, 2, NMP), np_fp8)
    tailT = Tq[kl_t, :, 12, :]               # [p, a, m]
    for q in range(NPAIR):
        tft_np[q, :, 0, :NM] = tailT[:, 2 * q]
        if 2 * q + 1 < KK:
            tft_np[q, :, 1, :NM] = tailT[:, 2 * q + 1]

    m_idx = np.arange(NM)
    ch_idx = m_idx // 36
    rem = m_idx % 36
    i_idx = np.arange(S_OUT)
    j_idx = np.arange(S_OUT)
    feat = (ch_idx[:, None, None] * 1296 + i_idx[None, :, None] * 216
            + j_idx[None, None, :] * 36 + rem[:, None, None])
    wl_np = (Wlin[0, feat].reshape(NM, S_OUT * S_OUT)
             / WSCALE).astype(np_bf16)

    bias4_np = np.ascontiguousarray(
        (b4[m_idx // 36] * WSCALE).astype(np.float32).reshape(NM, 1))
    blin_np = np.asarray(blin, np.float32).reshape(1, 1)
    return xm_all, xtl_all, tfm_np, tft_np, wl_np, bias4_np, blin_np


def kernel(x, W4, b4, Wlin, blin, _profile=False):
    x = np.asarray(x)
    W4 = np.asarray(W4)
    b4 = np.asarray(b4)
    Wlin = np.asarray(Wlin)
    blin = np.asarray(blin)

    (xm_all, xtl_all, tfm_np, tft_np, wl_np, bias4_np,
     blin_np) = _prep_inputs(x, W4, b4, Wlin, blin)

    if "nc" not in _CACHE:
        _CACHE["nc"] = _build_nc()
    nc = _CACHE["nc"]

    in_maps = []
    for core in range(N_CORES):
        b0 = core * B_CORE
        xmc = xm_all[..., b0:b0 + B_CORE].reshape(
            S_IN, 128, NU, 2, S_OUT, N_SUB, B_SUB)
        xmc = np.ascontiguousarray(xmc.transpose(5, 0, 1, 2, 3, 4, 6))
        xtc = xtl_all[..., b0:b0 + B_CORE].reshape(
            S_IN, NTAIL, 2, S_OUT, N_SUB, B_SUB)
        xtc = np.ascontiguousarray(xtc.transpose(4, 0, 1, 2, 3, 5))
        in_maps.append({
            "xm": xmc,
            "xtl": xtc,
            "tfm": tfm_np,
            "tft": tft_np,
            "wl": wl_np,
            "bias4": bias4_np,
            "blin": blin_np,
        })

    res = run_bass_kernel_spmd(
        nc, in_maps, core_ids=list(range(N_CORES)), trace=_profile)
    outs = [res.results[i]["out"].reshape(B_CORE) for i in range(N_CORES)]
    full = np.concatenate(outs).reshape(B_TOTAL, 1).astype(np.float32)
    if _profile:
        return full, res
    return full


# revision 5
# speedup vs baseline: 2.2636x; 1.0354x over previous
"""Trainium2 Bass kernel for nn_ModelSimplest (4D conv -> relu -> linear -> sigmoid).

fp8 DoubleRow, folded-boff + wavefront ramp + a-paired tails + tuned DMA order.

Per (a, oi): 4212 contraction rows r = (boff, k, l), J-shift baked into SBUF
tiles.  16 full 256-row DR matmuls (u<16) + (even a) one 116x2-row DR tail
matmul pairing (a, a+1) -> 215 matmuls per (t, oi), 384 cols each.

x tile per (t, ia): [128, 17, 2, 6, 64] fp8 — u<16 main units
(rows r = u*256 + g*128 + p), u=16 = tail block (p<116: g=0 rows
(boff=12, kl=208+p) of ia, g=1 same rows of ia+1; zero above).
tfm[a]: [128, 17, 2, 112] — u16 (even a) = tail-pair stationary.

Schedule: wavefront ramp over cells a+oi<5, then a-major main loop.  DMA
issue order keeps all ramp stationaries ahead of later x tiles.
"""
import sys
from contextlib import ExitStack

import numpy as np

sys.path.insert(0, "/opt/trn_rl_repo")

from concourse import bacc, bass, mybir, tile  # noqa: E402
from concourse.bass_utils import run_bass_kernel_spmd  # noqa: E402

KK = 13
S_IN = 18
S_OUT = 6
N_CORES = 8
B_TOTAL = 1024
B_CORE = B_TOTAL // N_CORES
B_SUB = 64
N_SUB = B_CORE // B_SUB
NCH = 3
NM = NCH * S_OUT * S_OUT              # 108
NMP = 112
NROW = KK * S_IN * S_IN               # 4212
NU = 16
NUX = 17                              # 16 main units + tail slot
NTAIL = NROW - NU * 256               # 116
WSCALE = 256.0
NSLOT = 10
RAMP_W = 5

F32 = mybir.dt.float32
BF16 = mybir.dt.bfloat16
FP8 = mybir.dt.float8e4
DR = mybir.MatmulPerfMode.DoubleRow

_CACHE = {}


def _build_nc():
    nc = bacc.Bacc(None, target_bir_lowering=False)

    xf = nc.dram_tensor("xf", [N_SUB, S_IN, 128, NUX, 2, S_OUT, B_SUB], FP8,
                        kind="ExternalInput")
    tfm = nc.dram_tensor("tfm", [KK, 128, NUX, 2, NMP], FP8,
                         kind="ExternalInput")
    wl = nc.dram_tensor("wl", [NM, S_OUT * S_OUT], BF16, kind="ExternalInput")
    bias4 = nc.dram_tensor("bias4", [NM, 1], F32, kind="ExternalInput")
    blin = nc.dram_tensor("blin", [1, 1], F32, kind="ExternalInput")
    out = nc.dram_tensor("out", [1, B_CORE], F32, kind="ExternalOutput")

    with tile.TileContext(nc) as tc, ExitStack() as ctx:
        cpool = ctx.enter_context(tc.tile_pool(name="consts", bufs=1))
        wl_sb = cpool.tile([NM, S_OUT * S_OUT], BF16)
        bias_sb = cpool.tile([NM, 1], F32)
        blin_sb = cpool.tile([1, 1], F32)
        consts_loaded = []

        def load_consts():
            if not consts_loaded:
                nc.sync.dma_start(wl_sb[:], wl[:])
                nc.sync.dma_start(bias_sb[:], bias4[:])
                nc.sync.dma_start(blin_sb[:], blin[:])
                consts_loaded.append(True)

        xpool = ctx.enter_context(tc.tile_pool(name="xs", bufs=1))
        twpool = ctx.enter_context(tc.tile_pool(name="tws", bufs=1))
        tw_tiles = {}

        def get_tw(a):
            if a not in tw_tiles:
                twt = twpool.tile([128, NUX, 2, NMP], FP8, tag=f"tfm{a}",
                                  name=f"tfm{a}")
                nc.sync.dma_start(twt[:], tfm[a])
                tw_tiles[a] = twt
            return tw_tiles[a]

        x_tiles = {}

        def load_x(t, ia, split=False):
            if (t, ia) in x_tiles:
                return
            xt = xpool.tile([128, NUX, 2, S_OUT, B_SUB], FP8,
                            tag=f"x{ia % NSLOT}", name=f"x_{t}_{ia}")
            if split:
                nc.sync.dma_start(xt[:, 0:6], xf[t, ia, :, 0:6])
                nc.sync.dma_start(xt[:, 6:NUX], xf[t, ia, :, 6:NUX])
            else:
                nc.sync.dma_start(xt[:], xf[t, ia])
            x_tiles[(t, ia)] = xt

        pspool = ctx.enter_context(
            tc.tile_pool(name="ps", bufs=1, space=bass.MemorySpace.PSUM))
        hpool = ctx.enter_context(tc.tile_pool(name="hs", bufs=1))
        opool = ctx.enter_context(tc.tile_pool(name="outs", bufs=2))

        pending = []

        def emit_epilogue():
            te, pse = pending.pop(0)
            lg = pspool.tile([1, B_SUB], F32, tag="lg", name=f"lg_{te}")
            for i in range(S_OUT):
                h = hpool.tile([NM, S_OUT, B_SUB], BF16, tag=f"h{i}",
                               name=f"h{i}_{te}")
                nc.scalar.activation(
                    h[:], pse[i][:],
                    mybir.ActivationFunctionType.Relu,
                    bias=bias_sb[:],
                )
                for j in range(S_OUT):
                    nc.tensor.matmul(
                        lg[:],
                        wl_sb[:, i * S_OUT + j:i * S_OUT + j + 1],
                        h[:, j, :],
                        start=(i == 0 and j == 0),
                        stop=(i == S_OUT - 1 and j == S_OUT - 1),
                    )
            ot = opool.tile([1, B_SUB], F32, tag="ot", name=f"ot_{te}")
            nc.scalar.activation(
                ot[:], lg[:],
                mybir.ActivationFunctionType.Sigmoid,
                bias=blin_sb[:],
            )
            nc.sync.dma_start(out[:, te * B_SUB:(te + 1) * B_SUB], ot[:])

        def cell(t, a, oi, ps):
            xt = x_tiles[(t, a + oi)]
            twt = tw_tiles[a]
            for u in range(NU):
                nc.tensor.matmul(
                    ps[oi][:],
                    twt[:, u, :, 0:NM],
                    xt[:, u, :, :, :],
                    start=(a == 0 and u == 0),
                    stop=False,
                    perf_mode=DR,
                )
            if a % 2 == 0:
                nc.tensor.matmul(
                    ps[oi][:],
                    twt[0:NTAIL, NU, :, 0:NM],
                    xt[0:NTAIL, NU, :, :, :],
                    start=False,
                    stop=(a == KK - 1),
                    perf_mode=DR,
                )

        for t in range(N_SUB):
            ps = [
                pspool.tile([NM, S_OUT, B_SUB], F32, tag=f"ps{i}",
                            name=f"ps{i}_{t}")
                for i in range(S_OUT)
            ]
            # DMA issue order: critical path first, all ramp stationaries
            # ahead of the later x tiles.
            get_tw(0)
            load_x(t, 0, split=(t == 0))
            get_tw(1)
            for ia in range(1, S_OUT):
                load_x(t, ia)
            for a in range(2, S_OUT):
                get_tw(a)
            load_consts()
            for ia in range(S_OUT, NSLOT):
                load_x(t, ia)
            # wavefront ramp
            for w in range(RAMP_W):
                for a in range(w + 1):
                    cell(t, a, w - a, ps)
                if w == 4 and pending:
                    emit_epilogue()
            # main loop
            for a in range(KK):
                get_tw(a)
                if a + 1 < KK:
                    get_tw(a + 1)
                nxt = a - 1 + NSLOT
                if a >= 1:
                    if nxt < S_IN:
                        load_x(t, nxt)
                    elif t + 1 < N_SUB:
                        load_x(t + 1, nxt - S_IN)
                for oi in range(S_OUT):
                    if a + oi >= RAMP_W:
                        cell(t, a, oi, ps)
            pending.append((t, ps))

        while pending:
            emit_epilogue()

    nc.compile()
    return nc


try:
    import ml_dtypes
    np_bf16 = ml_dtypes.bfloat16
    np_fp8 = ml_dtypes.float8_e4m3
except ImportError:  # pragma: no cover
    raise


def _prep_inputs(x, W4, b4, Wlin, blin):
    B = x.shape[0]
    r_main = np.arange(NU * 256).reshape(NU, 2, 128)
    boff_m = r_main // 324
    kl_m = r_main % 324
    kl_t = 208 + np.arange(NTAIL)

    xt = np.ascontiguousarray(
        x[:, 0].transpose(3, 4, 1, 2, 0)).reshape(324, S_IN, S_IN, B)
    xt8 = xt.astype(np_fp8)

    # main units: [u, g, p, j, ia, B] -> [ia, p, u, g, j, B]
    jj = boff_m[..., None] + np.arange(S_OUT)
    xm_all = xt8[kl_m[..., None], :, jj, :]
    xm_all = np.ascontiguousarray(xm_all.transpose(4, 2, 0, 1, 3, 5))

    # tail block: [ia, p, g, j, B]
    base = xt8[kl_t, :, 12:12 + S_OUT, :].transpose(1, 0, 2, 3)
    xtl_all = np.zeros((S_IN, NTAIL, 2, S_OUT, B), np_fp8)
    xtl_all[:, :, 0] = base
    xtl_all[:S_IN - 1, :, 1] = base[1:]

    # combined xf [ia, p, u(17), g, j, B]
    xf_all = np.zeros((S_IN, 128, NUX, 2, S_OUT, B), np_fp8)
    xf_all[:, :, :NU] = xm_all
    xf_all[:, :NTAIL, NU] = xtl_all

    T_flat = np.zeros((324, KK, KK, NM), np.float32)
    kl = np.arange(324)
    k_in_v = kl // S_IN
    l_in_v = kl % S_IN
    W4t = W4[:, 0].transpose(0, 3, 4, 1, 2)
    for ch in range(NCH):
        for kp in range(S_OUT):
            for lp in range(S_OUT):
                m = ch * 36 + kp * 6 + lp
                dk = k_in_v - kp
                dl = l_in_v - lp
                valid = (dk >= 0) & (dk < KK) & (dl >= 0) & (dl < KK)
                T_flat[valid, :, :, m] = W4t[ch, dk[valid], dl[valid]]
    Tq = (T_flat * WSCALE).astype(np_fp8)

    tfm_np = np.zeros((KK, 128, NUX, 2, NMP), np_fp8)
    tgt = Tq[kl_m, :, boff_m, :]             # [u, g, p, a, m]
    tfm_np[:, :, :NU, :, :NM] = tgt.transpose(3, 2, 0, 1, 4)
    tailT = Tq[kl_t, :, 12, :]               # [p, a, m]
    for a in range(0, KK, 2):
        tfm_np[a, :NTAIL, NU, 0, :NM] = tailT[:, a]
        if a + 1 < KK:
            tfm_np[a, :NTAIL, NU, 1, :NM] = tailT[:, a + 1]

    m_idx = np.arange(NM)
    ch_idx = m_idx // 36
    rem = m_idx % 36
    i_idx = np.arange(S_OUT)
    j_idx = np.arange(S_OUT)
    feat = (ch_idx[:, None, None] * 1296 + i_idx[None, :, None] * 216
            + j_idx[None, None, :] * 36 + rem[:, None, None])
    wl_np = (Wlin[0, feat].reshape(NM, S_OUT * S_OUT)
             / WSCALE).astype(np_bf16)

    bias4_np = np.ascontiguousarray(
        (b4[m_idx // 36] * WSCALE).astype(np.float32).reshape(NM, 1))
    blin_np = np.asarray(blin, np.float32).reshape(1, 1)
    return xf_all, tfm_np, wl_np, bias4_np, blin_np


def kernel(x, W4, b4, Wlin, blin, _profile=False):
    x = np.asarray(x)
    W4 = np.asarray(W4)
    b4 = np.asarray(b4)
    Wlin = np.asarray(Wlin)
    blin = np.asarray(blin)

    xf_all, tfm_np, wl_np, bias4_np, blin_np = _prep_inputs(
        x, W4, b4, Wlin, blin)

    if "nc" not in _CACHE:
        _CACHE["nc"] = _build_nc()
    nc = _CACHE["nc"]

    in_maps = []
    for core in range(N_CORES):
        b0 = core * B_CORE
        xc = xf_all[..., b0:b0 + B_CORE].reshape(
            S_IN, 128, NUX, 2, S_OUT, N_SUB, B_SUB)
        xc = np.ascontiguousarray(xc.transpose(5, 0, 1, 2, 3, 4, 6))
        in_maps.append({
            "xf": xc,
            "tfm": tfm_np,
            "wl": wl_np,
            "bias4": bias4_np,
            "blin": blin_np,
        })

    res = run_bass_kernel_spmd(
        nc, in_maps, core_ids=list(range(N_CORES)), trace=_profile)
    outs = [res.results[i]["out"].reshape(B_CORE) for i in range(N_CORES)]
    full = np.concatenate(outs).reshape(B_TOTAL, 1).astype(np.float32)
    if _profile:
        return full, res
    return full
